# revision 1
# baseline (speedup 1.0000x reference)
"""ConvFormer Trainium2 kernel — data-parallel over B across 8 NeuronCores.

Reference (per batch element b):
    x1 = x[b].T                                   # (D, L) channel-major
    for (W, bias) in ((W3,b3),(W5,b5),(W7,b7)):   # chained masked convs
        x1 = bias + sum_k mask_k * (W[:,:,k] @ shift(x1, k))
    h  = LN(x[b] + x1.T)                          # (L, D)
    out = LN(h + gelu(h @ w1 + b1) @ w2 + b2)

Device strategy (per core):
  - masked conv via masked-input trick: Z_d = x_shifted_d * M_d (mask depends
    only on output position -> factors onto the moving/stationary operand),
    so all (tap, c-chunk) matmuls accumulate in PSUM with zero DVE between.
  - convs 1,2: channel-major -> channel-major (PSUM out [o,l]), bounced
    through HBM (SBUF cannot hold full-L stages + weights).
  - conv 3: channel-major -> L-major (PSUM out [l,o]) which lands directly in
    LayerNorm layout; residual (x + b7, host-folded) fused into the copyout.
  - LN via bn_stats/bn_aggr (D=512 == BN_STATS_FMAX).
  - MLP: PE-transpose h -> hc per l-chunk, matmuls with float32r, gelu fused
    into the PSUM->SBUF copyout on ScalarE (bias b1 fused too), transpose
    back fused with the residual add.
  - all matmuls in float32r: full PE rate (1 cyc/row at N=512), ~1.5e-4 err.
"""

import numpy as np

B, L, D = 8, 4096, 512
KS = (3, 5, 7)
EPS = 1e-5
NCORES = 8
PAD = 4            # zero-pad columns on each side of channel-major tensors
BLK = 1024         # conv L-block size
NBLK = L // BLK
CD = D // 128      # 4 channel chunks
LCH = L // 128     # 32 L-chunks of 128
H = 4 * D          # mlp hidden
JD = H // 128      # 16 hidden chunks
GELU_FUNC_NAME = "Gelu_apprx_tanh"  # jax.nn.gelu default is approximate=True

_CACHE = {}


def _build_nc(ln1_affine, ln2_affine, b2_nonzero):
    import concourse.bass as bass
    import concourse.tile as tile
    from concourse import bacc, mybir
    from concourse.masks import make_identity

    f32 = mybir.dt.float32
    f32r = mybir.dt.float32r
    bf16 = mybir.dt.bfloat16
    GELU = getattr(mybir.ActivationFunctionType, GELU_FUNC_NAME)
    COPY = mybir.ActivationFunctionType.Copy
    SQRT = mybir.ActivationFunctionType.Sqrt
    ADD = mybir.AluOpType.add
    SUB = mybir.AluOpType.subtract
    MULT = mybir.AluOpType.mult

    PL = PAD + L + PAD
    # global tap -> (conv index, shift d); convs use taps [0:3], [3:8], [8:15]
    conv_taps = []
    t0 = 0
    for ki, K in enumerate(KS):
        p = (K - 1) // 2
        conv_taps.append([(t0 + i, i - p) for i in range(K)])
        t0 += K
    NT = t0  # 15
    # mask row index for shift d (d != 0)
    d2m = {-3: 0, -2: 1, -1: 2, 1: 3, 2: 4, 3: 5}

    nc = bacc.Bacc(None, target_bir_lowering=False)

    xcp = nc.declare_dram_parameter("xcp", [CD, 128, PL], f32r, isOutput=False)
    xb = nc.declare_dram_parameter("xb", [LCH, 128, D], f32, isOutput=False)
    masks = nc.declare_dram_parameter("masks", [6, L], bf16, isOutput=False)
    wc = nc.declare_dram_parameter("wc", [128, NT * CD * D], f32r, isOutput=False)
    cb1 = nc.declare_dram_parameter("cb1", [CD, 128], f32, isOutput=False)
    cb2 = nc.declare_dram_parameter("cb2", [CD, 128], f32, isOutput=False)
    w1 = nc.declare_dram_parameter("w1", [128, CD * H], f32r, isOutput=False)
    b1c = nc.declare_dram_parameter("b1c", [JD, 128], f32, isOutput=False)
    w2 = nc.declare_dram_parameter("w2", [128, JD * D], f32r, isOutput=False)
    b2c = nc.declare_dram_parameter("b2c", [CD, 128], f32, isOutput=False)
    if ln1_affine:
        g1r = nc.declare_dram_parameter("g1r", [1, D], f32, isOutput=False)
        b1r = nc.declare_dram_parameter("b1r", [1, D], f32, isOutput=False)
    if ln2_affine:
        g2r = nc.declare_dram_parameter("g2r", [1, D], f32, isOutput=False)
        b2r = nc.declare_dram_parameter("b2r", [1, D], f32, isOutput=False)
    out = nc.declare_dram_parameter("out", [L, D], f32, isOutput=True)

    # HBM intermediates: conv stage outputs (padded channel-major), h tiles
    x2p = nc.dram_tensor("x2p", [CD, 128, PL], f32r)
    x3p = nc.dram_tensor("x3p", [CD, 128, PL], f32r)
    hbuf = nc.dram_tensor("hbuf", [LCH, 128, D], f32r)

    def bcast_row_ap(param, row, col0, n):
        """DMA access pattern: one DRAM row slice broadcast to 128 partitions."""
        src = param[row, col0:col0 + n]
        return bass.AP(tensor=src.tensor, offset=src.offset, ap=[[0, 128]] + list(src.ap))

    with tile.TileContext(nc) as tc:
        with tc.tile_pool(name="const", bufs=1) as const:
            ident32 = const.tile([128, 128], f32)
            make_identity(nc, ident32)
            ident = const.tile([128, 128], f32r)
            nc.vector.tensor_copy(out=ident[:], in_=ident32[:])
            epst = const.tile([128, 1], f32)
            nc.vector.memset(epst, EPS)
            zpad32 = const.tile([128, PAD], f32)
            nc.vector.memset(zpad32, 0.0)
            zpad = const.tile([128, PAD], f32r)
            nc.vector.tensor_copy(out=zpad[:], in_=zpad32[:])
            cb1t = const.tile([128, CD], f32)
            nc.sync.dma_start(out=cb1t[:], in_=cb1.rearrange("c p -> p c"))
            cb2t = const.tile([128, CD], f32)
            nc.sync.dma_start(out=cb2t[:], in_=cb2.rearrange("c p -> p c"))
            b1ct = const.tile([128, JD], f32)
            nc.sync.dma_start(out=b1ct[:], in_=b1c.rearrange("j p -> p j"))
            if b2_nonzero:
                b2ct = const.tile([128, CD], f32)
                nc.sync.dma_start(out=b2ct[:], in_=b2c.rearrange("c p -> p c"))
            if ln1_affine:
                g1t = const.tile([128, D], f32)
                nc.sync.dma_start(out=g1t[:], in_=bcast_row_ap(g1r, 0, 0, D))
                b1t = const.tile([128, D], f32)
                nc.sync.dma_start(out=b1t[:], in_=bcast_row_ap(b1r, 0, 0, D))
            if ln2_affine:
                g2t = const.tile([128, D], f32)
                nc.sync.dma_start(out=g2t[:], in_=bcast_row_ap(g2r, 0, 0, D))
                b2t = const.tile([128, D], f32)
                nc.sync.dma_start(out=b2t[:], in_=bcast_row_ap(b2r, 0, 0, D))


            # ---------------- Phase 1: conv chain + LN1 -> hbuf ----------------
            with (
                tc.tile_pool(name="wconv", bufs=2) as wpool,
                tc.tile_pool(name="inblk", bufs=6) as inpool,
                tc.tile_pool(name="maskp", bufs=4) as mpool,
                tc.tile_pool(name="zp", bufs=6) as zpool,
                tc.tile_pool(name="cout", bufs=4) as copool,
                tc.tile_pool(name="xbp", bufs=3) as xbpool,
                tc.tile_pool(name="sp", bufs=3) as spool,
                tc.tile_pool(name="stats", bufs=8) as stpool,
                tc.tile_pool(name="hp", bufs=3) as hpool,
                tc.tile_pool(name="psum1", bufs=8, space="PSUM") as pspool,
            ):
                for ci, (K, taps) in enumerate(zip(KS, conv_taps)):
                    src = (xcp, x2p, x3p)[ci]
                    dst = (x2p, x3p, None)[ci]
                    if ci < 2:
                        # zero the pad columns of this conv's output before
                        # the next conv's halo reads
                        for c in range(CD):
                            nc.sync.dma_start(out=dst[c, :, 0:PAD], in_=zpad[:])
                            nc.sync.dma_start(out=dst[c, :, PAD + L:PL], in_=zpad[:])
                    wt = wpool.tile([128, K * CD * D], f32r, tag="wconv")
                    for ti in range(K):
                        eng = nc.scalar if ti % 2 == 0 else nc.sync
                        a0 = (taps[0][0] + ti) * CD * D
                        eng.dma_start(
                            out=wt[:, ti * CD * D:(ti + 1) * CD * D],
                            in_=wc[:, a0:a0 + CD * D],
                        )
                    for blk in range(NBLK):
                        l0 = blk * BLK
                        # input tiles with +-3 halo (block-local col i maps to l0+i-3)
                        ints = []
                        for c in range(CD):
                            it = inpool.tile([128, BLK + 6], f32r, tag="inblk")
                            nc.sync.dma_start(
                                out=it[:], in_=src[c, :, PAD + l0 - 3:PAD + l0 + BLK + 3]
                            )
                            ints.append(it)
                        # All 8 PSUM banks hold this block's output tiles; taps
                        # accumulate tap-major so only one tap's masked inputs
                        # (CD tiles) are live at a time.
                        nps = (CD * (BLK // 512)) if ci < 2 else (BLK // 128)
                        pss = [
                            pspool.tile([128, 512], f32, tag="ps", name=f"ps{i}")
                            for i in range(nps)
                        ]
                        for ti, (t, d) in enumerate(taps):
                            if d == 0:
                                zcs = [ints[c][:, 3:3 + BLK] for c in range(CD)]
                            else:
                                mt = mpool.tile([128, BLK], bf16, tag="maskp")
                                nc.scalar.dma_start(
                                    out=mt[:], in_=bcast_row_ap(masks, d2m[d], l0, BLK)
                                )
                                zcs = []
                                for c in range(CD):
                                    zt = zpool.tile([128, BLK], f32r, tag="zp")
                                    nc.vector.tensor_tensor(
                                        out=zt[:],
                                        in0=ints[c][:, 3 + d:3 + d + BLK],
                                        in1=mt[:],
                                        op=MULT,
                                    )
                                    zcs.append(zt)
                            for c in range(CD):
                                kw = dict(
                                    start=(ti == 0 and c == 0),
                                    stop=(ti == K - 1 and c == CD - 1),
                                    skip_group_check=True,
                                )
                                if ci < 2:
                                    # CM conv: out[o,l]; lhsT = W[c,o]; rhs = Z[c,l]
                                    for o in range(CD):
                                        for lc in range(BLK // 512):
                                            nc.tensor.matmul(
                                                pss[o * (BLK // 512) + lc][:],
                                                wt[:, (ti * CD + c) * D + o * 128:(ti * CD + c) * D + (o + 1) * 128],
                                                zcs[c][:, lc * 512:(lc + 1) * 512],
                                                **kw,
                                            )
                                else:
                                    # LM conv: out[l,o]; lhsT = Z[c,l-slice]; rhs = W[c,o]
                                    for lch in range(BLK // 128):
                                        nc.tensor.matmul(
                                            pss[lch][:],
                                            zcs[c][:, lch * 128:(lch + 1) * 128],
                                            wt[:, (ti * CD + c) * D:(ti * CD + c + 1) * D],
                                            **kw,
                                        )

                        if ci < 2:
                            bct = cb1t if ci == 0 else cb2t
                            for o in range(CD):
                                for lc in range(BLK // 512):
                                    ps = pss[o * (BLK // 512) + lc]
                                    ot = copool.tile([128, 512], f32r, tag="cout")
                                    nc.vector.tensor_scalar(
                                        out=ot[:], in0=ps[:],
                                        scalar1=bct[:, o:o + 1], scalar2=None, op0=ADD,
                                    )
                                    nc.sync.dma_start(
                                        out=dst[o, :, PAD + l0 + lc * 512:PAD + l0 + (lc + 1) * 512],
                                        in_=ot[:],
                                    )
                        else:
                            for lch in range(BLK // 128):
                                lg = (l0 + lch * 128) // 128  # global l-chunk
                                ps = pss[lch]
                                # s = conv_out + (x + b7) ; then LN1
                                xbt = xbpool.tile([128, D], f32, tag="xbp")
                                nc.sync.dma_start(out=xbt[:], in_=xb[lg])
                                st = spool.tile([128, D], f32, tag="sp")
                                nc.vector.scalar_tensor_tensor(
                                    out=st[:], in0=ps[:], scalar=1.0, in1=xbt[:],
                                    op0=MULT, op1=ADD,
                                )
                                stats = stpool.tile([128, 6], f32, tag="st6")
                                nc.vector.bn_stats(out=stats[:], in_=st[:])
                                mv = stpool.tile([128, 2], f32, tag="mv")
                                nc.vector.bn_aggr(out=mv[:], in_=stats[:])
                                std = stpool.tile([128, 1], f32, tag="sd")
                                nc.scalar.activation(
                                    out=std[:], in_=mv[:, 1:2], func=SQRT,
                                    bias=epst[:], scale=1.0,
                                )
                                nc.vector.reciprocal(out=std[:], in_=std[:])
                                ht = hpool.tile([128, D], f32r, tag="hp")
                                nc.vector.tensor_scalar(
                                    out=ht[:], in0=st[:],
                                    scalar1=mv[:, 0:1], scalar2=std[:],
                                    op0=SUB, op1=MULT,
                                )
                                if ln1_affine:
                                    nc.vector.tensor_tensor(out=ht[:], in0=ht[:], in1=g1t[:], op=MULT)
                                    nc.vector.tensor_tensor(out=ht[:], in0=ht[:], in1=b1t[:], op=ADD)
                                nc.sync.dma_start(out=hbuf[lg], in_=ht[:])

            # ---------------- Phase 2: MLP + LN2 -> out ----------------
            with (
                tc.tile_pool(name="wmlp", bufs=1) as wmpool,
                tc.tile_pool(name="hin", bufs=8) as hinpool,
                tc.tile_pool(name="hcp", bufs=2) as hcpool,
                tc.tile_pool(name="hid", bufs=JD + 2) as hidpool,
                tc.tile_pool(name="msb", bufs=8) as msbpool,
                tc.tile_pool(name="s2p", bufs=6) as s2pool,
                tc.tile_pool(name="st2", bufs=8) as st2pool,
                tc.tile_pool(name="outp", bufs=4) as outpool,
                tc.tile_pool(name="psA", bufs=2, space="PSUM") as psA,
                tc.tile_pool(name="psB", bufs=2, space="PSUM") as psB,
                tc.tile_pool(name="psT", bufs=2, space="PSUM") as psT,
                tc.tile_pool(name="psU", bufs=2, space="PSUM") as psU,
            ):
                w1t = wmpool.tile([128, CD * H], f32r, tag="w1")
                for q in range(4):
                    eng = nc.scalar if q % 2 == 0 else nc.sync
                    eng.dma_start(out=w1t[:, q * H:(q + 1) * H],
                                  in_=w1[:, q * H:(q + 1) * H])
                w2t = wmpool.tile([128, JD * D], f32r, tag="w2")
                for q in range(4):
                    eng = nc.scalar if q % 2 == 0 else nc.sync
                    eng.dma_start(out=w2t[:, q * 4 * D:(q + 1) * 4 * D],
                                  in_=w2[:, q * 4 * D:(q + 1) * 4 * D])

                for lg in range(L // 512):
                    hts = []
                    for i in range(4):
                        ht = hinpool.tile([128, D], f32r, tag="hin")
                        nc.sync.dma_start(out=ht[:], in_=hbuf[lg * 4 + i])
                        hts.append(ht)
                    # hc[d][:, i*128:(i+1)*128] = h_i[:, d*128:(d+1)*128].T
                    hct = hcpool.tile([128, CD, 512], f32r, tag="hcp")
                    for d in range(CD):
                        pt = psT.tile([128, 512], f32r, tag="psT")
                        for i in range(4):
                            nc.tensor.transpose(
                                pt[:, i * 128:(i + 1) * 128],
                                hts[i][:, d * 128:(d + 1) * 128],
                                ident[:],
                            )
                        nc.vector.tensor_copy(out=hct[:, d], in_=pt[:])
                    # MLP1 + gelu (bias fused on ScalarE)
                    hids = []
                    for j in range(JD):
                        ps = psA.tile([128, 512], f32, tag="psA")
                        for d in range(CD):
                            nc.tensor.matmul(
                                ps[:],
                                w1t[:, d * H + j * 128:d * H + (j + 1) * 128],
                                hct[:, d],
                                start=(d == 0),
                                stop=(d == CD - 1),
                            )
                        hj = hidpool.tile([128, 512], f32r, tag="hid")
                        nc.scalar.activation(
                            out=hj[:], in_=ps[:], func=GELU,
                            bias=b1ct[:, j:j + 1], scale=1.0,
                        )
                        hids.append(hj)
                    # MLP2
                    msbs = []
                    for o in range(CD):
                        ps = psB.tile([128, 512], f32, tag="psB")
                        for j in range(JD):
                            nc.tensor.matmul(
                                ps[:],
                                w2t[:, j * D + o * 128:j * D + (o + 1) * 128],
                                hids[j][:],
                                start=(j == 0),
                                stop=(j == JD - 1),
                            )
                        mo = msbpool.tile([128, 512], f32r, tag="msb")
                        if b2_nonzero:
                            nc.vector.tensor_scalar(
                                out=mo[:], in0=ps[:],
                                scalar1=b2ct[:, o:o + 1], scalar2=None, op0=ADD,
                            )
                        else:
                            nc.scalar.activation(out=mo[:], in_=ps[:], func=COPY)
                        msbs.append(mo)
                    # transpose back + residual + LN2 + store
                    for i in range(4):
                        s2 = s2pool.tile([128, D], f32, tag="s2p")
                        for o in range(CD):
                            pt = psU.tile([128, 128], f32r, tag="psU")
                            nc.tensor.transpose(
                                pt[:], msbs[o][:, i * 128:(i + 1) * 128], ident[:]
                            )
                            nc.vector.tensor_tensor(
                                out=s2[:, o * 128:(o + 1) * 128],
                                in0=pt[:], in1=hts[i][:, o * 128:(o + 1) * 128], op=ADD,
                            )
                        stats = st2pool.tile([128, 6], f32, tag="st6b")
                        nc.vector.bn_stats(out=stats[:], in_=s2[:])
                        mv = st2pool.tile([128, 2], f32, tag="mvb")
                        nc.vector.bn_aggr(out=mv[:], in_=stats[:])
                        std = st2pool.tile([128, 1], f32, tag="sdb")
                        nc.scalar.activation(
                            out=std[:], in_=mv[:, 1:2], func=SQRT,
                            bias=epst[:], scale=1.0,
                        )
                        nc.vector.reciprocal(out=std[:], in_=std[:])
                        ot = outpool.tile([128, D], f32, tag="outp")
                        nc.vector.tensor_scalar(
                            out=ot[:], in0=s2[:],
                            scalar1=mv[:, 0:1], scalar2=std[:],
                            op0=SUB, op1=MULT,
                        )
                        if ln2_affine:
                            nc.vector.tensor_tensor(out=ot[:], in0=ot[:], in1=g2t[:], op=MULT)
                            nc.vector.tensor_tensor(out=ot[:], in0=ot[:], in1=b2t[:], op=ADD)
                        lr = (lg * 4 + i) * 128
                        nc.sync.dma_start(out=out[lr:lr + 128, :], in_=ot[:])

    nc.compile()
    return nc


def _prep_inputs(x, chain, W3, b3, W5, b5, W7, b7,
                 mlp_w1, mlp_b1, mlp_w2, mlp_b2,
                 ln1_g, ln1_b, ln2_g, ln2_b):
    import ml_dtypes

    f32 = np.float32
    x = np.asarray(x, f32)
    chain = np.asarray(chain, np.int32)
    flags = (
        not (np.all(np.asarray(ln1_g) == 1.0) and np.all(np.asarray(ln1_b) == 0.0)),
        not (np.all(np.asarray(ln2_g) == 1.0) and np.all(np.asarray(ln2_b) == 0.0)),
        bool(np.any(np.asarray(mlp_b2) != 0.0)),
    )

    # conv weights: per global tap t -> W[:, :, kt].T  (shape [c, o])
    wct = np.empty((15, D, D), f32)
    t = 0
    for W in (W3, W5, W7):
        W = np.asarray(W, f32)
        for k in range(W.shape[2]):
            wct[t] = W[:, :, k].T
            t += 1
    # partition-major flat: wc[p, ((t*CD + c)*D + o)] = W_t[c*128+p, o]
    wc = np.ascontiguousarray(
        wct.reshape(15, CD, 128, D).transpose(2, 0, 1, 3).reshape(128, 15 * CD * D))

    shared = {
        "wc": wc,
        "cb1": np.asarray(b3, f32).reshape(CD, 128),
        "cb2": np.asarray(b5, f32).reshape(CD, 128),
        "w1": np.ascontiguousarray(np.asarray(mlp_w1, f32).reshape(CD, 128, H).transpose(1, 0, 2).reshape(128, CD * H)),
        "b1c": np.asarray(mlp_b1, f32).reshape(JD, 128),
        "w2": np.ascontiguousarray(np.asarray(mlp_w2, f32).reshape(JD, 128, D).transpose(1, 0, 2).reshape(128, JD * D)),
        "b2c": np.asarray(mlp_b2, f32).reshape(CD, 128),
    }
    if flags[0]:
        shared["g1r"] = np.asarray(ln1_g, f32).reshape(1, D)
        shared["b1r"] = np.asarray(ln1_b, f32).reshape(1, D)
    if flags[1]:
        shared["g2r"] = np.asarray(ln2_g, f32).reshape(1, D)
        shared["b2r"] = np.asarray(ln2_b, f32).reshape(1, D)

    b7f = np.asarray(b7, f32)
    in_maps = []
    for b in range(B):
        xc = x[b].T  # (D, L)
        xcp = np.zeros((CD, 128, PAD + L + PAD), f32)
        xcp[:, :, PAD:PAD + L] = xc.reshape(CD, 128, L)
        xbv = (x[b] + b7f[None, :]).reshape(LCH, 128, D)
        # masks for shifts d in (-3,-2,-1,1,2,3), evaluated at output position
        ce = np.zeros(L + 8, np.int32)
        ce[4:4 + L] = chain[b]
        m = np.empty((6, L), ml_dtypes.bfloat16)
        for mi, d in enumerate((-3, -2, -1, 1, 2, 3)):
            m[mi] = (ce[4 + d:4 + d + L] == chain[b]).astype(ml_dtypes.bfloat16)
        im = {"xcp": xcp, "xb": np.ascontiguousarray(xbv),
              "masks": m, **shared}
        in_maps.append(im)
    return in_maps, flags


def kernel(**inputs):
    from concourse.bass_utils import run_bass_kernel_spmd

    in_maps, flags = _prep_inputs(**inputs)
    if flags not in _CACHE:
        _CACHE[flags] = _build_nc(*flags)
    nc = _CACHE[flags]
    res = run_bass_kernel_spmd(nc, in_maps, list(range(NCORES)))
    return np.stack([res.results[b]["out"] for b in range(B)]).astype(np.float32)



# revision 2
# speedup vs baseline: 1.0039x; 1.0039x over previous
"""ConvFormer Trainium2 kernel v2 — fused single-pass, bf16 matmul operands.

Data-parallel over B across 8 NeuronCores (batch element b -> core b).

Per core:
    x1 = x.T (channel-major); 3 chained masked convs (K=3,5,7) where each
    tap's contribution is masked by (chain[l+d]==chain[l]); h = LN(x + x1.T);
    out = LN(h + gelu(h@w1+b1)@w2 + b2).

v2 design (vs v1 two-phase f32r kernel at ~1.01 ms):
  - all matmul operands bf16 (same 1 cyc/row PE rate as f32r, half the
    SBUF/DMA footprint, 2x DVE rate for mask multiplies). PSUM stays f32.
  - conv stage outputs s1, s2 are SBUF-resident (no HBM bounce) -> single
    fused pass, PE stream never waits on HBM roundtrips.
  - stage 3 emitted L-major so conv out lands directly in LayerNorm layout;
    residual (x + b7, host-folded) read as xb.
  - MLP fused per 512-token block right after stage-3/LN1 of that block;
    mlp2 emitted L-major (no transpose-back, LN2 reads PSUM directly).
  - LN rsqrt via DVE tensor_scalar((var+eps), pow -0.5) -> the Act engine
    only ever runs Copy/Gelu (one table, no ACT_TABLE_LOAD thrash).
"""

import numpy as np

B, L, D = 8, 4096, 512
KS = (3, 5, 7)
EPS = 1e-5
NCORES = 8
PAD = 4            # zero-pad cols each side of channel-major tensors
PL = PAD + L + PAD
NBLK = L // 512    # 8 blocks of 512
CD = D // 128      # 4 channel chunks
LCH = L // 128     # 32 L-chunks of 128
H = 4 * D          # mlp hidden
JD = H // 128      # 16 hidden chunks
NT = sum(KS)       # 15 taps total
GELU_FUNC_NAME = "Gelu_apprx_tanh"  # jax.nn.gelu default is approximate=True

_CACHE = {}


def _build_nc(ln1_affine, ln2_affine, b2_nonzero):
    import concourse.bass as bass
    import concourse.tile as tile
    from concourse import bacc, mybir
    from concourse.masks import make_identity

    f32 = mybir.dt.float32
    bf16 = mybir.dt.bfloat16
    GELU = getattr(mybir.ActivationFunctionType, GELU_FUNC_NAME)
    IDENT = mybir.ActivationFunctionType.Identity
    ADD = mybir.AluOpType.add
    SUB = mybir.AluOpType.subtract
    MULT = mybir.AluOpType.mult
    POW = mybir.AluOpType.pow

    # global tap -> (conv index, shift d); convs use taps [0:3], [3:8], [8:15]
    conv_taps = []
    t0 = 0
    for K in KS:
        p = (K - 1) // 2
        conv_taps.append([(t0 + i, i - p) for i in range(K)])
        t0 += K
    d2m = {-3: 0, -2: 1, -1: 2, 1: 3, 2: 4, 3: 5}

    nc = bacc.Bacc(None, target_bir_lowering=False)

    xcb = nc.declare_dram_parameter("xcb", [CD, 128, PL], bf16, isOutput=False)
    xb = nc.declare_dram_parameter("xb", [LCH, 128, D], f32, isOutput=False)
    masks = nc.declare_dram_parameter("masks", [6, L], bf16, isOutput=False)
    wc = nc.declare_dram_parameter("wc", [128, NT * CD * D], bf16, isOutput=False)
    cb1 = nc.declare_dram_parameter("cb1", [CD, 128], f32, isOutput=False)
    cb2 = nc.declare_dram_parameter("cb2", [CD, 128], f32, isOutput=False)
    w1 = nc.declare_dram_parameter("w1", [128, CD * H], bf16, isOutput=False)
    b1c = nc.declare_dram_parameter("b1c", [JD, 128], f32, isOutput=False)
    w2 = nc.declare_dram_parameter("w2", [128, JD * D], bf16, isOutput=False)
    if b2_nonzero:
        b2r = nc.declare_dram_parameter("b2r", [1, D], f32, isOutput=False)
    if ln1_affine:
        g1r = nc.declare_dram_parameter("g1r", [1, D], f32, isOutput=False)
        b1r = nc.declare_dram_parameter("b1r", [1, D], f32, isOutput=False)
    if ln2_affine:
        g2r = nc.declare_dram_parameter("g2r", [1, D], f32, isOutput=False)
        b2lr = nc.declare_dram_parameter("b2lr", [1, D], f32, isOutput=False)
    out = nc.declare_dram_parameter("out", [L, D], f32, isOutput=True)

    def bcast_row_ap(param, row, col0, n):
        """DMA access pattern: one DRAM row slice broadcast to 128 partitions."""
        src = param[row, col0:col0 + n]
        return bass.AP(tensor=src.tensor, offset=src.offset, ap=[[0, 128]] + list(src.ap))

    with tile.TileContext(nc) as tc:
        with (
            tc.tile_pool(name="const", bufs=1) as const,
            tc.tile_pool(name="wcp", bufs=1) as wcp,
            tc.tile_pool(name="s2p", bufs=1) as s2pool,
        ):
            ident32 = const.tile([128, 128], f32)
            make_identity(nc, ident32)
            identb = const.tile([128, 128], bf16)
            nc.vector.tensor_copy(out=identb[:], in_=ident32[:])
            cb1t = const.tile([128, CD], f32)
            nc.sync.dma_start(out=cb1t[:], in_=cb1.rearrange("c p -> p c"))
            cb2t = const.tile([128, CD], f32)
            nc.sync.dma_start(out=cb2t[:], in_=cb2.rearrange("c p -> p c"))
            b1ct = const.tile([128, JD], f32)
            nc.sync.dma_start(out=b1ct[:], in_=b1c.rearrange("j p -> p j"))
            if b2_nonzero:
                b2t = const.tile([128, D], f32)
                nc.sync.dma_start(out=b2t[:], in_=bcast_row_ap(b2r, 0, 0, D))
            if ln1_affine:
                g1t = const.tile([128, D], f32)
                nc.sync.dma_start(out=g1t[:], in_=bcast_row_ap(g1r, 0, 0, D))
                b1t = const.tile([128, D], f32)
                nc.sync.dma_start(out=b1t[:], in_=bcast_row_ap(b1r, 0, 0, D))
            if ln2_affine:
                g2t = const.tile([128, D], f32)
                nc.sync.dma_start(out=g2t[:], in_=bcast_row_ap(g2r, 0, 0, D))
                b2lt = const.tile([128, D], f32)
                nc.sync.dma_start(out=b2lt[:], in_=bcast_row_ap(b2lr, 0, 0, D))

            wct = wcp.tile([128, NT * CD * D], bf16)
            # per-stage chunks so stage-1 matmuls start after a 1.6MB load
            tb = [0, 3, 8, 15]
            for q in range(3):
                a0, a1 = tb[q] * CD * D, tb[q + 1] * CD * D
                eng = nc.scalar if q % 2 == 0 else nc.sync
                eng.dma_start(out=wct[:, a0:a1], in_=wc[:, a0:a1])

            s2t = s2pool.tile([128, CD, PL], bf16)
            for c in range(CD):
                nc.vector.memset(s2t[:, c, 0:PAD], 0.0)
                nc.vector.memset(s2t[:, c, PAD + L:PL], 0.0)

            def masked_input(zpool, srcf, mt_cache, d, width=512, tag="zp"):
                """One tap: list of 4 c-chunk [128,width] bf16 APs (masked)."""
                if d == 0:
                    return [srcf(c, 0) for c in range(CD)]
                mt = mt_cache[d]
                zcs = []
                for c in range(CD):
                    zt = zpool.tile([128, width], bf16, tag=tag)
                    nc.vector.tensor_tensor(
                        out=zt[:], in0=srcf(c, d), in1=mt[:], op=MULT)
                    zcs.append(zt[:])
                return zcs

            def load_masks(mpool, taps, l0, eng, width=512, tag="maskp"):
                mts = {}
                for (t, d) in taps:
                    if d == 0:
                        continue
                    mt = mpool.tile([128, width], bf16, tag=tag)
                    eng.dma_start(out=mt[:], in_=bcast_row_ap(masks, d2m[d], l0, width))
                    mts[d] = mt
                return mts

            # ================= stage 1 + stage 2 (channel-major) =================
            with (
                tc.tile_pool(name="s1p", bufs=1) as s1pool,
                tc.tile_pool(name="xin", bufs=8) as xin,
                tc.tile_pool(name="maskA", bufs=12) as mpoolA,
                tc.tile_pool(name="zpA", bufs=8) as zpoolA,
                tc.tile_pool(name="cps", bufs=8, space="PSUM") as cps,
            ):
                s1t = s1pool.tile([128, CD, PL], bf16)
                for c in range(CD):
                    nc.vector.memset(s1t[:, c, 0:PAD], 0.0)
                    nc.vector.memset(s1t[:, c, PAD + L:PL], 0.0)

                for stage in (0, 1):
                    K = KS[stage]
                    taps = conv_taps[stage]
                    p = (K - 1) // 2
                    dstt, bct = (s1t, cb1t) if stage == 0 else (s2t, cb2t)
                    for blk in range(NBLK):
                        l0 = blk * 512
                        base = PAD + l0
                        if stage == 0:
                            xts = []
                            for c in range(CD):
                                xt = xin.tile([128, 512 + 2 * p], bf16, tag="xin")
                                nc.sync.dma_start(
                                    out=xt[:], in_=xcb[c, :, base - p:base + 512 + p])
                                xts.append(xt)
                            srcf = lambda c, d: xts[c][:, p + d:p + d + 512]
                        else:
                            srcf = lambda c, d: s1t[:, c, base + d:base + d + 512]
                        mts = load_masks(mpoolA, taps, l0, nc.gpsimd)
                        pss = [cps.tile([128, 512], f32, tag="cps", name=f"cps{o}") for o in range(CD)]
                        for ti, (t, d) in enumerate(taps):
                            zcs = masked_input(zpoolA, srcf, mts, d)
                            for c in range(CD):
                                for o in range(CD):
                                    a0 = (t * CD + c) * D + o * 128
                                    nc.tensor.matmul(
                                        pss[o][:],
                                        wct[:, a0:a0 + 128],
                                        zcs[c],
                                        start=(ti == 0 and c == 0),
                                        stop=(ti == K - 1 and c == CD - 1),
                                        skip_group_check=True,
                                    )
                        for o in range(CD):
                            nc.scalar.activation(
                                out=dstt[:, o, base:base + 512], in_=pss[o][:],
                                func=IDENT, bias=bct[:, o:o + 1], scale=1.0)

            # ============ stage 3 (L-major) + LN1 + MLP + LN2, fused ============
            with (
                tc.tile_pool(name="wm", bufs=1) as wm,
                tc.tile_pool(name="xbp", bufs=4) as xbp,
                tc.tile_pool(name="stp", bufs=4) as stp,
                tc.tile_pool(name="stats", bufs=10) as statp,
                tc.tile_pool(name="hbfp", bufs=10) as hbfp,
                tc.tile_pool(name="hctp", bufs=2) as hctp,
                tc.tile_pool(name="hidp", bufs=4) as hidp,
                tc.tile_pool(name="st2p", bufs=4) as st2p,
                tc.tile_pool(name="otp", bufs=4) as otp,
                tc.tile_pool(name="mask3", bufs=12) as mpoolB,
                tc.tile_pool(name="zp3", bufs=8) as zpoolB,
                tc.tile_pool(name="psum", bufs=5, space="PSUM") as psp,
            ):
                w1t = wm.tile([128, CD * H], bf16)
                w2t = wm.tile([128, JD * D], bf16)
                for q in range(2):
                    h0 = q * CD * H // 2
                    nc.scalar.dma_start(out=w1t[:, h0:h0 + CD * H // 2],
                                        in_=w1[:, h0:h0 + CD * H // 2])
                    d0 = q * JD * D // 2
                    nc.sync.dma_start(out=w2t[:, d0:d0 + JD * D // 2],
                                      in_=w2[:, d0:d0 + JD * D // 2])

                i32 = mybir.dt.int32
                SHR = mybir.AluOpType.logical_shift_right

                def rsqrt_var(v_ap):
                    """rs = 1/sqrt(v+eps) on DVE [128,1]: quake seed + 2 Newton."""
                    vt = statp.tile([128, 1], f32, tag="vt")
                    nc.vector.tensor_scalar(
                        out=vt[:], in0=v_ap, scalar1=EPS, scalar2=None, op0=ADD)
                    y0b = statp.tile([128, 1], i32, tag="y0b")
                    nc.vector.tensor_scalar(
                        out=y0b[:], in0=vt[:].bitcast(i32), scalar1=1,
                        scalar2=None, op0=SHR)
                    nc.vector.tensor_scalar(
                        out=y0b[:], in0=y0b[:], scalar1=-1, scalar2=0x5F3759DF,
                        op0=MULT, op1=ADD)
                    cur = y0b[:].bitcast(f32)
                    for it in range(2):
                        aq = statp.tile([128, 1], f32, tag=f"nta{it}")
                        nc.vector.tensor_tensor(out=aq[:], in0=cur, in1=cur, op=MULT)
                        nc.vector.tensor_tensor(out=aq[:], in0=aq[:], in1=vt[:], op=MULT)
                        nc.vector.tensor_scalar(
                            out=aq[:], in0=aq[:], scalar1=-0.5, scalar2=1.5,
                            op0=MULT, op1=ADD)
                        yn = statp.tile([128, 1], f32, tag=f"nty{it}")
                        nc.vector.tensor_tensor(out=yn[:], in0=cur, in1=aq[:], op=MULT)
                        cur = yn[:]
                    return cur

                taps = conv_taps[2]
                K = KS[2]
                NB2 = L // 256

                def conv3_mm(blk):
                    l0 = blk * 256
                    base = PAD + l0
                    srcf = lambda c, d: s2t[:, c, base + d:base + d + 256]
                    mts = load_masks(mpoolB, taps, l0, nc.gpsimd, width=256, tag="mask3")
                    st3 = [psp.tile([128, 512], f32, tag="acc", name=f"st3_{i}") for i in range(2)]
                    for ti, (t, d) in enumerate(taps):
                        zcs = masked_input(zpoolB, srcf, mts, d, width=256, tag="zp3")
                        for c in range(CD):
                            a0 = (t * CD + c) * D
                            for i in range(2):
                                nc.tensor.matmul(
                                    st3[i][:],
                                    zcs[c][:, i * 128:(i + 1) * 128],
                                    wct[:, a0:a0 + D],
                                    start=(ti == 0 and c == 0),
                                    stop=(ti == K - 1 and c == CD - 1),
                                    skip_group_check=True,
                                )
                    return st3

                def drain3(blk, st3):
                    # residual add straight out of PSUM -> frees st3 banks early
                    sts = []
                    for i in range(2):
                        lg = blk * 2 + i
                        xbt = xbp.tile([128, D], f32, tag="xbp")
                        nc.gpsimd.dma_start(out=xbt[:], in_=xb[lg])
                        st = stp.tile([128, D], f32, tag="stp")
                        nc.vector.scalar_tensor_tensor(
                            out=st[:], in0=st3[i][:], scalar=1.0, in1=xbt[:],
                            op0=MULT, op1=ADD)
                        sts.append(st)
                    return sts

                def post(blk, sts):
                    # LN1 per 128-l chunk; h kept bf16 (matmul + residual reuse)
                    hbfs = []
                    for i in range(2):
                        st = sts[i]
                        stats = statp.tile([128, 6], f32, tag="st6")
                        nc.vector.bn_stats(out=stats[:], in_=st[:])
                        mv = statp.tile([128, 2], f32, tag="mv")
                        nc.vector.bn_aggr(out=mv[:], in_=stats[:])
                        rs = rsqrt_var(mv[:, 1:2])
                        hb = hbfp.tile([128, D], bf16, tag="hbf")
                        nc.vector.tensor_scalar(
                            out=hb[:], in0=st[:], scalar1=mv[:, 0:1], scalar2=rs,
                            op0=SUB, op1=MULT)
                        if ln1_affine:
                            nc.vector.tensor_tensor(out=hb[:], in0=hb[:], in1=g1t[:], op=MULT)
                            nc.vector.tensor_tensor(out=hb[:], in0=hb[:], in1=b1t[:], op=ADD)
                        hbfs.append(hb)
                    # transpose h -> hct (channel-major) for mlp1; all four
                    # d-chunks packed into one PSUM bank
                    hct = hctp.tile([128, CD, 256], bf16, tag="hct")
                    pt_all = psp.tile([128, CD, 256], bf16, tag="psT", bufs=1)
                    for dd in range(CD):
                        for i in range(2):
                            nc.tensor.transpose(
                                pt_all[:, dd, i * 128:(i + 1) * 128],
                                hbfs[i][:, dd * 128:(dd + 1) * 128],
                                identb[:],
                            )
                    nc.vector.tensor_copy(out=hct[:], in_=pt_all[:])
                    # mlp1 (hidden chunks in pairs per PSUM bank) + gelu,
                    # then mlp2 (L-major) accumulating into psB
                    psB = [psp.tile([128, 512], f32, tag="acc", name=f"psB{i}") for i in range(2)]
                    for jp in range(JD // 2):
                        psa = psp.tile([128, 512], f32, tag="psA", bufs=2)
                        for jj in range(2):
                            j = jp * 2 + jj
                            for dd in range(CD):
                                nc.tensor.matmul(
                                    psa[:, jj * 256:(jj + 1) * 256],
                                    w1t[:, dd * H + j * 128:dd * H + (j + 1) * 128],
                                    hct[:, dd],
                                    start=(dd == 0),
                                    stop=(dd == CD - 1),
                                    skip_group_check=True,
                                )
                        hjs = []
                        for jj in range(2):
                            j = jp * 2 + jj
                            hj = hidp.tile([128, 256], bf16, tag="hid")
                            nc.scalar.activation(
                                out=hj[:], in_=psa[:, jj * 256:(jj + 1) * 256],
                                func=GELU, bias=b1ct[:, j:j + 1], scale=1.0)
                            hjs.append(hj)
                        for jj in range(2):
                            j = jp * 2 + jj
                            for i in range(2):
                                nc.tensor.matmul(
                                    psB[i][:],
                                    hjs[jj][:, i * 128:(i + 1) * 128],
                                    w2t[:, j * D:(j + 1) * D],
                                    start=(j == 0),
                                    stop=(j == JD - 1),
                                    skip_group_check=True,
                                )
                    # LN2 per 128-l chunk, straight from PSUM
                    for i in range(2):
                        st2 = st2p.tile([128, D], f32, tag="st2")
                        nc.vector.scalar_tensor_tensor(
                            out=st2[:], in0=psB[i][:], scalar=1.0, in1=hbfs[i][:],
                            op0=MULT, op1=ADD)
                        if b2_nonzero:
                            nc.vector.tensor_tensor(out=st2[:], in0=st2[:], in1=b2t[:], op=ADD)
                        stats = statp.tile([128, 6], f32, tag="st6")
                        nc.vector.bn_stats(out=stats[:], in_=st2[:])
                        mv = statp.tile([128, 2], f32, tag="mv")
                        nc.vector.bn_aggr(out=mv[:], in_=stats[:])
                        rs = rsqrt_var(mv[:, 1:2])
                        ot = otp.tile([128, D], f32, tag="otp")
                        nc.vector.tensor_scalar(
                            out=ot[:], in0=st2[:], scalar1=mv[:, 0:1], scalar2=rs,
                            op0=SUB, op1=MULT)
                        if ln2_affine:
                            nc.vector.tensor_tensor(out=ot[:], in0=ot[:], in1=g2t[:], op=MULT)
                            nc.vector.tensor_tensor(out=ot[:], in0=ot[:], in1=b2lt[:], op=ADD)
                        lr = (blk * 2 + i) * 128
                        nc.sync.dma_start(out=out[lr:lr + 128, :], in_=ot[:])

                # software pipeline: conv matmuls + PSUM drain of block b+1
                # are emitted before block b's LN/MLP so the PE never waits
                # on the LN1 dependency chain and st3 banks free early.
                prev = drain3(0, conv3_mm(0))
                for blk in range(NB2):
                    nxt = drain3(blk + 1, conv3_mm(blk + 1)) if blk + 1 < NB2 else None
                    post(blk, prev)
                    prev = nxt

    nc.compile()
    return nc


def _prep_inputs(x, chain, W3, b3, W5, b5, W7, b7,
                 mlp_w1, mlp_b1, mlp_w2, mlp_b2,
                 ln1_g, ln1_b, ln2_g, ln2_b):
    import ml_dtypes

    f32 = np.float32
    bf = ml_dtypes.bfloat16
    x = np.asarray(x, f32)
    chain = np.asarray(chain, np.int32)
    flags = (
        not (np.all(np.asarray(ln1_g) == 1.0) and np.all(np.asarray(ln1_b) == 0.0)),
        not (np.all(np.asarray(ln2_g) == 1.0) and np.all(np.asarray(ln2_b) == 0.0)),
        bool(np.any(np.asarray(mlp_b2) != 0.0)),
    )

    # conv weights: per global tap t -> W[:, :, kt].T  (shape [c, o])
    wct = np.empty((NT, D, D), f32)
    t = 0
    for W in (W3, W5, W7):
        W = np.asarray(W, f32)
        for k in range(W.shape[2]):
            wct[t] = W[:, :, k].T
            t += 1
    # partition-major flat: wc[p, ((t*CD + c)*D + o)] = W_t[c*128+p, o]
    wc = np.ascontiguousarray(
        wct.reshape(NT, CD, 128, D).transpose(2, 0, 1, 3).reshape(128, NT * CD * D)
    ).astype(bf)

    shared = {
        "wc": wc,
        "cb1": np.asarray(b3, f32).reshape(CD, 128),
        "cb2": np.asarray(b5, f32).reshape(CD, 128),
        "w1": np.ascontiguousarray(
            np.asarray(mlp_w1, f32).reshape(CD, 128, H).transpose(1, 0, 2)
            .reshape(128, CD * H)).astype(bf),
        "b1c": np.asarray(mlp_b1, f32).reshape(JD, 128),
        "w2": np.ascontiguousarray(
            np.asarray(mlp_w2, f32).reshape(JD, 128, D).transpose(1, 0, 2)
            .reshape(128, JD * D)).astype(bf),
    }
    if flags[0]:
        shared["g1r"] = np.asarray(ln1_g, f32).reshape(1, D)
        shared["b1r"] = np.asarray(ln1_b, f32).reshape(1, D)
    if flags[1]:
        shared["g2r"] = np.asarray(ln2_g, f32).reshape(1, D)
        shared["b2lr"] = np.asarray(ln2_b, f32).reshape(1, D)
    if flags[2]:
        shared["b2r"] = np.asarray(mlp_b2, f32).reshape(1, D)

    b7f = np.asarray(b7, f32)
    in_maps = []
    for b in range(B):
        xc = x[b].T  # (D, L)
        xcp = np.zeros((CD, 128, PL), f32)
        xcp[:, :, PAD:PAD + L] = xc.reshape(CD, 128, L)
        xbv = (x[b] + b7f[None, :]).reshape(LCH, 128, D)
        # masks for shifts d in (-3,-2,-1,1,2,3), evaluated at output position
        ce = np.zeros(L + 8, np.int32)
        ce[4:4 + L] = chain[b]
        m = np.empty((6, L), bf)
        for mi, d in enumerate((-3, -2, -1, 1, 2, 3)):
            m[mi] = (ce[4 + d:4 + d + L] == chain[b]).astype(bf)
        im = {"xcb": xcp.astype(bf), "xb": np.ascontiguousarray(xbv),
              "masks": m, **shared}
        in_maps.append(im)
    return in_maps, flags


def kernel(**inputs):
    from concourse.bass_utils import run_bass_kernel_spmd

    in_maps, flags = _prep_inputs(**inputs)
    if flags not in _CACHE:
        _CACHE[flags] = _build_nc(*flags)
    nc = _CACHE[flags]
    res = run_bass_kernel_spmd(nc, in_maps, list(range(NCORES)))
    return np.stack([res.results[b]["out"] for b in range(B)]).astype(np.float32)
